# revision 64
# baseline (speedup 1.0000x reference)
"""AttnBlock (GroupNorm + single-head self-attention + proj + residual) on 8 trn2 cores.

Sharding: core = (batch b = core//4, query-block qb = core%4). Each core gets its
batch's x rolled so its 1024 queries are columns 0:1024; attention key/value
order is permutation-invariant so the roll is free. No cross-core communication.

Math (v6, fp8 DoubleRow attention core):
  GroupNorm folded into per-channel affine A, B: hn = A*x + B.
  Logits fold ("M-trick"): logits[j,i] = x[:,j]^T A (wk^T wq) A x[:,i] + gamma[j]
    MTraw = wq^T wk (bf16, weights only -- computed during the x DMA);
    MT1 = A_q*MTraw rounded to fp8 pairs; qk = A_k*(MT1^T-pairs @ x_fp8) + A_k*c0
    c0[ck] = sum_cq MTraw[cq,ck] B[cq] + (wk^T bq)[ck] (k-bias drops by softmax
    shift invariance; the q-bias survives as this per-key offset).
  P = exp(logits/sqrt(C) - 2) unnormalized (the -2 keeps P <= ~40, safely inside
  fp8e4m3 range; the shift cancels in the normalization). o = (wv*A@x) @ P with
  the division by column sums applied at the projection output.
  v/o biases collapse to bo'' = wo@(wv@B + bv) + bo, pre-added to the residual.

fp8 DoubleRow: the four big matmul streams (qk, vT, logits, o) run with
e4m3 operands and perf_mode=DoubleRow -- 256-deep contraction per call at 0.5
PE cycles/row, halving tensor-engine time vs bf16. Pair layout is slot-major
blocks ([p, (two n)]), so every producer (ACT exp, DVE scalar ops) writes a
contiguous slab; no extra interleave passes exist anywhere.
Accuracy: M/proj matmuls stay bf16; all PSUM accumulation f32; softmax sums
f32; residual f32. The fp8 rounding lands on tensors whose error averages out
over the 4096-key softmax (measured: see test.py).

Schedule: 11 DMA descriptors (issue cost ~620ns each on Sync), ordered
x_fp8 -> wq|wk|woT -> vecs -> wv_fp8 -> residual. GroupNorm stats split
DVE (bn_stats, 5/8) vs ACT (Square/Copy+accum, 3/8). Warm matmuls bridge the
two DMA waits so the HAM clock never drops (idle >2us halves the PE clock for
~20us). vT-phase PSUM->SBUF copies alternate DVE/ACT.
"""

import numpy as np
import ml_dtypes

import concourse.bass as bass
import concourse.bacc as bacc
import concourse.tile as tile
from concourse import mybir
from concourse.bass_utils import run_bass_kernel_spmd

F32 = mybir.dt.float32
F32R = mybir.dt.float32r
BF16 = mybir.dt.bfloat16
FP8 = mybir.dt.float8e4
AF = mybir.ActivationFunctionType
ALU = mybir.AluOpType
AX = mybir.AxisListType
DR = mybir.MatmulPerfMode.DoubleRow

B, C, HH, WW = 2, 512, 64, 64
N = HH * WW          # 4096 pixels
NQ = N // 4          # queries per core
G = 32               # groups
GPT = 8              # groups per 128-channel tile
NT = C // 128        # 4 channel blocks
NG = 2               # fp8 pair groups (256 channels each)
JT = N // 128        # 32 key tiles
JP = JT // 2         # 16 key pair-tiles
CW = 512             # query chunk width
NCH = NQ // CW       # 2 chunks per core
EPS = 1e-6
SCALE = float(C) ** -0.5
ESHIFT = -2.0        # exp(s*logit + ESHIFT): keeps P within fp8e4m3 range
BSC = 16.0           # B is scaled by 16 into fp8 so it sits in normal range

_CACHE: dict = {}


def _f32(ap):
    return ap.bitcast(F32)


def _pairs(tile_ap):
    return tile_ap.rearrange("p (two n) -> p two n", two=2)


def _build_bass():
    nc = bacc.Bacc("TRN2")

    xp8_d = nc.declare_dram_parameter("xp8", [2 * 128, 2 * N], FP8, isOutput=False)
    wb_d = nc.declare_dram_parameter("wb", [C, 3 * C], BF16, isOutput=False)
    vb_d = nc.declare_dram_parameter("vb", [128, 28], F32, isOutput=False)
    selT_d = nc.declare_dram_parameter("selT", [GPT, 128], F32, isOutput=False)
    wv8_d = nc.declare_dram_parameter("wv8", [2 * 128, 2 * C], FP8, isOutput=False)
    xr_d = nc.declare_dram_parameter("xr", [128, NT * NQ], BF16, isOutput=False)
    out_d = nc.declare_dram_parameter("out", [C, NQ], F32, isOutput=True)

    dram = dict(xp8=xp8_d, wb=wb_d, vb=vb_d, selT=selT_d, wv8=wv8_d, xr=xr_d,
                out=out_d)
    with tile.TileContext(nc) as tc, \
         nc.allow_low_precision(reason="fp8/bf16 PE inputs with f32 accumulation"):
        _emit(tc, {k: v.ap() for k, v in dram.items()})
    nc.compile()
    return nc


def _emit(tc, d):
    nc = tc.nc

    # ---- long-lived pools -------------------------------------------------
    xp = tc.alloc_tile_pool(name="xp", bufs=NG)
    rp = tc.alloc_tile_pool(name="rp", bufs=1)
    wbp = tc.alloc_tile_pool(name="wbp", bufs=NT)
    wv8p = tc.alloc_tile_pool(name="wv8p", bufs=NG)
    mp = tc.alloc_tile_pool(name="mp", bufs=NT + NG)  # MTraw bf16 + MT1 fp8
    vecs = tc.alloc_tile_pool(name="vecs", bufs=1)
    qkp = tc.alloc_tile_pool(name="qkp", bufs=NCH * NG)
    vtp = tc.alloc_tile_pool(name="vtp", bufs=JP)

    # ---- DMA in (11 issues; issue order ~ arrival order) ------------------
    # x in fp8 pair layout: tile g row p col (s*N + n) = x[256g+128s+p, n]
    # one DMA per slab (slot-major half): per-queue bandwidth is the limiter,
    # and the finer grain lets each slab's stats start on its own completion
    xg = d["xp8"].rearrange("(g p) m -> g p m", p=128)
    x8_sb = []
    for g in range(NG):
        xt = xp.tile([128, 2 * N], FP8, tag="x8", name=f"x8_{g}")
        for s in range(2):
            nc.sync.dma_start(out=xt[:, s * N:(s + 1) * N],
                              in_=xg[g][:, s * N:(s + 1) * N])
        x8_sb.append(xt)

    vb_sb = vecs.tile([128, 28], F32, tag="vb")
    nc.sync.dma_start(out=vb_sb[:, :], in_=d["vb"])
    gnw_sb = vb_sb[:, 0:4]
    gnb_sb = vb_sb[:, 4:8]
    bvv_sb = vb_sb[:, 8:12]
    bov_sb = vb_sb[:, 12:16]
    bqf_sb = vb_sb[:, 16:20]
    sel_sb = vb_sb[:, 20:28]
    selT_sb = vecs.tile([GPT, 128], F32, tag="selT")
    nc.sync.dma_start(out=selT_sb[:, :], in_=d["selT"])

    wb_sb = [wbp.tile([128, 3 * C], BF16, tag="wb", name=f"wb{t}")
             for t in range(NT)]
    wq_sb = [w[:, 0:C] for w in wb_sb]
    wk_sb = [w[:, C:2 * C] for w in wb_sb]
    woT_sb = [w[:, 2 * C:3 * C] for w in wb_sb]
    wv8_sb = [wv8p.tile([128, 2 * C], FP8, tag="wv8", name=f"wv8_{g}")
              for g in range(NG)]
    xrt = rp.tile([128, NT * NQ], BF16, tag="xr")
    xr_sb = [xrt[:, t * NQ:(t + 1) * NQ] for t in range(NT)]

    # ---- SBUF constants ---------------------------------------------------
    warm_sb = vecs.tile([128, CW], BF16, tag="warm")
    nc.vector.memset(warm_sb[:, :], 0.0)
    ones32_sb = vecs.tile([128, 128], F32, tag="ones32")
    nc.vector.memset(ones32_sb[:, :], 1.0)
    ones128_sb = vecs.tile([128, 128], F32R, tag="ones128")
    nc.vector.tensor_copy(out=ones128_sb[:, :], in_=ones32_sb[:, :])
    eshift_sb = vecs.tile([128, 1], F32, tag="eshift")
    nc.vector.memset(eshift_sb[:, :], ESHIFT)

    # (measured: x's slab arrival is a DMA-engine floor, ~0.2MB/us for the
    # fp8 stream regardless of contention — gating the transfers below on
    # x's first slab gained nothing and delayed the weights)
    wb_t = d["wb"].rearrange("(t p) m -> t p m", p=128)
    for t in range(NT):
        nc.sync.dma_start(out=wb_sb[t][:, :], in_=wb_t[t])
    wv8g = d["wv8"].rearrange("(g p) m -> g p m", p=128)
    for g in range(NG):
        nc.sync.dma_start(out=wv8_sb[g][:, :], in_=wv8g[g])
    nc.sync.dma_start(out=xrt[:, :], in_=d["xr"])

    A_sb = vecs.tile([128, NT], F32, tag="A")
    B_sb = vecs.tile([128, NT], F32, tag="B")
    Bb_sb = vecs.tile([128, NT], BF16, tag="Bb")      # B bf16 for c0 rhs
    B8_sb = vecs.tile([128, NT], FP8, tag="B8")       # 16*B fp8 for bv' rhs
    bqv_sb = vecs.tile([128, NT], BF16, tag="bqv")    # bq bf16
    c0A_sb = vecs.tile([128, NT], F32, tag="c0A")
    bvp_sb = vecs.tile([128, NT], F32, tag="bvp")
    bvpb_sb = vecs.tile([128, NT], BF16, tag="bvpb")
    bop_sb = vecs.tile([128, NT], F32, tag="bop")
    nc.vector.tensor_copy(out=bqv_sb[:, :], in_=bqf_sb)

    # ---- prologue: warm-up + M_raw + GroupNorm stats ----------------------
    with tc.tile_pool(name="stp", bufs=4) as stp, \
         tc.tile_pool(name="pspro", bufs=2, space="PSUM") as ps_pro:
        nwarm = [0]

        def emit_warm(n):
            for _ in range(n):
                wt = ps_pro.tile([128, CW], F32, tag="pro", name=f"wm{nwarm[0]}")
                nwarm[0] += 1
                nc.tensor.matmul(out=wt[:, :], lhsT=warm_sb[:, 0:128],
                                 rhs=warm_sb[:, :], start=True, stop=True)

        emit_warm(46)

        # MTraw[cq, ck] = sum_o wq[o, cq] wk[o, ck]  (weights only, no x/A dep)
        mt_sb = []
        for cq in range(NT):
            mps = ps_pro.tile([128, C], F32, tag="pro", name=f"mps{cq}")
            for ot in range(NT):
                nc.tensor.matmul(out=mps[:, :],
                                 lhsT=wq_sb[ot][:, cq * 128:(cq + 1) * 128],
                                 rhs=wk_sb[ot][:, :],
                                 start=(ot == 0), stop=(ot == NT - 1))
            mt = mp.tile([128, C], BF16, tag="mt", name=f"mt{cq}")
            nc.scalar.copy(out=mt[:, :], in_=mps[:, :])
            mt_sb.append(mt)

        emit_warm(10)

        # stats per channel-block slab (g, s) as the x DMA lands, split:
        #   segs 0-4: DVE bn_stats -> [mean_a, var_a]
        #   segs 5-7: ACT Square+accum (sumsq_b), ACT Copy+accum (sum_b)
        # st2 rows: [mean_a, E2_a, sum_b, sumsq_b]; weights applied after the
        # group-select matmul on the [8,4] tile.
        NSA, NSB = 5 * 512, 3 * 512
        sqs = vecs.tile([128, NSB], BF16, tag="sqs")   # ACT scratch
        gps_t = []
        for bi in range(NT):
            g, s = bi // 2, bi % 2
            slab = x8_sb[g][:, s * N:(s + 1) * N]
            st = stp.tile([128, 5, 6], F32, tag="bnst", name=f"bnst{bi}")
            xr_ = slab[:, 0:NSA].rearrange("p (s n) -> p s n", s=5)
            st2 = stp.tile([128, 4], F32, tag="st2", name=f"st2_{bi}")
            nc.scalar.activation(out=sqs[:, :], in_=slab[:, NSA:N],
                                 func=AF.Square, bias=0.0, scale=1.0,
                                 accum_out=st2[:, 3:4])
            nc.scalar.activation(out=sqs[:, :], in_=slab[:, NSA:N],
                                 func=AF.Copy, scale=1.0,
                                 accum_out=st2[:, 2:3])
            for q in range(5):
                nc.vector.bn_stats(out=st[:, q, :], in_=xr_[:, q, :])
            mv = stp.tile([128, 2], F32, tag="mv", name=f"mv{bi}")
            nc.vector.bn_aggr(out=mv[:, :], in_=st[:, :, :])
            nc.vector.tensor_copy(out=st2[:, 0:1], in_=mv[:, 0:1])
            nc.vector.tensor_mul(out=st2[:, 1:2], in0=mv[:, 0:1], in1=mv[:, 0:1])
            nc.vector.tensor_add(out=st2[:, 1:2], in0=st2[:, 1:2], in1=mv[:, 1:2])
            gps = ps_pro.tile([GPT, 4], F32, tag="gps", name=f"gps{bi}")
            nc.tensor.matmul(out=gps[:, :], lhsT=sel_sb, rhs=st2[:, :],
                             start=True, stop=True)
            gps_t.append(gps)
            emit_warm((3, 3, 3, 0)[bi])

        # group mean / rstd; DVE preps first, then batched ACT Sqrts
        GN = 16.0 * N
        grp_t = []
        for t in range(NT):
            grp = stp.tile([GPT, 2], F32, tag="grp", name=f"grp{t}")
            ga = stp.tile([GPT, 2], F32, tag="ga", name=f"ga{t}")
            nc.vector.tensor_scalar_mul(out=ga[:, :], in0=gps_t[t][:, 0:2],
                                        scalar1=NSA / GN)
            nc.vector.tensor_scalar_mul(out=grp[:, :], in0=gps_t[t][:, 2:4],
                                        scalar1=1.0 / GN)
            nc.vector.tensor_add(out=grp[:, :], in0=grp[:, :], in1=ga[:, :])
            gtmp = stp.tile([GPT, 1], F32, tag="gtmp", name=f"gtmp{t}")
            nc.vector.tensor_mul(out=gtmp[:, :], in0=grp[:, 0:1], in1=grp[:, 0:1])
            nc.vector.tensor_sub(out=grp[:, 1:2], in0=grp[:, 1:2], in1=gtmp[:, :])
            nc.vector.tensor_scalar_add(out=grp[:, 1:2], in0=grp[:, 1:2], scalar1=EPS)
            grp_t.append(grp)
        for t in range(NT):
            nc.scalar.activation(out=grp_t[t][:, 1:2], in_=grp_t[t][:, 1:2],
                                 func=AF.Sqrt, bias=0.0, scale=1.0)
        for t in range(NT):
            nc.vector.reciprocal(out=grp_t[t][:, 1:2], in_=grp_t[t][:, 1:2])
            mrp = ps_pro.tile([128, 2], F32, tag="sm", name=f"mrp{t}")
            nc.tensor.matmul(out=mrp[:, :], lhsT=selT_sb[:, :], rhs=grp_t[t][:, :],
                             start=True, stop=True)
            tcol = slice(t, t + 1)
            nc.vector.tensor_mul(out=A_sb[:, tcol], in0=gnw_sb[:, tcol], in1=mrp[:, 1:2])
            nc.vector.tensor_mul(out=B_sb[:, tcol], in0=mrp[:, 0:1], in1=A_sb[:, tcol])
            nc.vector.tensor_sub(out=B_sb[:, tcol], in0=gnb_sb[:, tcol], in1=B_sb[:, tcol])
            nc.vector.tensor_copy(out=Bb_sb[:, tcol], in_=B_sb[:, tcol])
            nc.vector.tensor_scalar_mul(out=B8_sb[:, tcol], in0=B_sb[:, tcol],
                                        scalar1=BSC)

        # MT1 fp8 pairs: slot (g, s) slab = A[2g+s] * MTraw[2g+s]
        mt1_sb = []
        for g in range(NG):
            m1 = mp.tile([128, 2 * C], FP8, tag="mt1", name=f"mt1_{g}")
            for s in range(2):
                cq = 2 * g + s
                nc.vector.tensor_scalar_mul(out=m1[:, s * C:(s + 1) * C],
                                            in0=mt_sb[cq][:, :],
                                            scalar1=A_sb[:, cq:cq + 1])
            mt1_sb.append(m1)

    ps_mm = tc.alloc_tile_pool(name="psmm", bufs=3, space="PSUM")
    nwarm2 = [0]

    def emit_warm2(n):
        for _ in range(n):
            wt = ps_mm.tile([128, CW], F32, tag="mm", name=f"w2_{nwarm2[0]}")
            nwarm2[0] += 1
            nc.tensor.matmul(out=wt[:, :], lhsT=warm_sb[:, 0:128],
                             rhs=warm_sb[:, :], start=True, stop=True)

    emit_warm2(8)

    # ---- c0A[ck] = A_k * (sum_cq MTraw[cq,ck] B[cq] + wk^T bq) ------------
    for ck in range(NT):
        cps = ps_mm.tile([128, 1], F32, tag="mm", name=f"c0{ck}")
        for cq in range(NT):
            nc.tensor.matmul(out=cps[:, :],
                             lhsT=mt_sb[cq][:, ck * 128:(ck + 1) * 128],
                             rhs=Bb_sb[:, cq:cq + 1],
                             start=(cq == 0), stop=False)
        for ot in range(NT):
            nc.tensor.matmul(out=cps[:, :],
                             lhsT=wk_sb[ot][:, ck * 128:(ck + 1) * 128],
                             rhs=bqv_sb[:, ot:ot + 1],
                             start=False, stop=(ot == NT - 1))
        nc.vector.tensor_mul(out=c0A_sb[:, ck:ck + 1], in0=cps[:, :],
                             in1=A_sb[:, ck:ck + 1])

    # ---- qk (both chunks, fp8 DoubleRow): A_k*(MT1^T @ x) + c0A -----------
    emit_warm2(8)
    qk_sb = [[None] * NG for _ in range(NCH)]
    for ch in range(NCH):
        csl = slice(ch * CW, (ch + 1) * CW)
        for ck in range(NT):
            kps = ps_mm.tile([128, CW], F32, tag="mm")
            for g in range(NG):
                nc.tensor.matmul(out=kps[:, :],
                                 lhsT=_pairs(mt1_sb[g])[:, :, ck * 128:(ck + 1) * 128],
                                 rhs=_pairs(x8_sb[g])[:, :, csl],
                                 start=(g == 0), stop=(g == NG - 1),
                                 perf_mode=DR)
            if qk_sb[ch][ck // 2] is None:
                qk_sb[ch][ck // 2] = qkp.tile([128, 2 * CW], FP8, tag="qk",
                                              name=f"qk{ch}_{ck // 2}")
            nc.vector.tensor_scalar(out=qk_sb[ch][ck // 2][:, (ck % 2) * CW:
                                                           (ck % 2 + 1) * CW],
                                    in0=kps[:, :],
                                    scalar1=A_sb[:, ck:ck + 1],
                                    scalar2=c0A_sb[:, ck:ck + 1],
                                    op0=ALU.mult, op1=ALU.add)

    # ---- folded v bias: bv' = wv@(16B)/16 + bv (unscaled fp8 wvT) ---------
    for ot in range(NT):
        ocol = slice(ot, ot + 1)
        bps2 = ps_mm.tile([128, 1], F32, tag="mm", name=f"bv{ot}")
        k = 0
        for g in range(NG):
            for s in range(2):
                nc.tensor.matmul(out=bps2[:, :],
                                 lhsT=wv8_sb[g][:, s * C + ot * 128:
                                                s * C + (ot + 1) * 128],
                                 rhs=B8_sb[:, 2 * g + s:2 * g + s + 1],
                                 start=(k == 0), stop=(k == 3))
                k += 1
        nc.vector.tensor_scalar(out=bvp_sb[:, ocol], in0=bps2[:, :],
                                scalar1=1.0 / BSC, scalar2=bvv_sb[:, ocol],
                                op0=ALU.mult, op1=ALU.add)
        nc.vector.tensor_copy(out=bvpb_sb[:, ocol], in_=bvp_sb[:, ocol])

    # ---- scale wv8 slabs by A (after the bias fold) -----------------------
    for g in range(NG):
        for s in range(2):
            nc.vector.tensor_scalar_mul(out=wv8_sb[g][:, s * C:(s + 1) * C],
                                        in0=wv8_sb[g][:, s * C:(s + 1) * C],
                                        scalar1=A_sb[:, 2 * g + s:2 * g + s + 1])

    ps_o = tc.alloc_tile_pool(name="pso", bufs=4, space="PSUM")
    # the 8th PSUM bank: a 4th rotating vT buffer so the PE never stalls on
    # the DVE/ACT vt-copy backlog (measured ~1.2us gap -> 6.8us half-clock)
    ps_x = tc.alloc_tile_pool(name="psx", bufs=1, space="PSUM")

    # ---- vT pairs: vt[jp] slot jt%2 = ((wv*A) @ x)^T[jt-block] ------------
    vt_sb = []
    for jt in range(JT):
        jsl = slice(jt * 128, (jt + 1) * 128)
        pool = ps_x if jt % 4 == 3 else ps_mm
        vps = pool.tile([128, C], F32, tag="mm" if pool is ps_mm else "vx",
                        name=f"vps{jt}")
        for g in range(NG):
            nc.tensor.matmul(out=vps[:, :],
                             lhsT=_pairs(x8_sb[g])[:, :, jsl],
                             rhs=_pairs(wv8_sb[g])[:, :, :],
                             start=(g == 0), stop=(g == NG - 1),
                             perf_mode=DR)
        if jt % 2 == 0:
            vt_sb.append(vtp.tile([128, 2 * C], FP8, tag="vt",
                                  name=f"vt{jt // 2}"))
        dst = vt_sb[jt // 2][:, (jt % 2) * C:(jt % 2 + 1) * C]
        if jt % 2:
            nc.scalar.copy(out=dst, in_=vps[:, :])
        else:
            nc.vector.tensor_copy(out=dst, in_=vps[:, :])
        if jt == 1:
            # bo'' = wo@bv' + bo, off the critical path once woT has landed
            for ot2 in range(NT):
                oc2 = slice(ot2, ot2 + 1)
                bps3 = ps_mm.tile([128, 1], F32, tag="mm", name=f"bo{ot2}")
                for ci2 in range(NT):
                    nc.tensor.matmul(out=bps3[:, :],
                                     lhsT=woT_sb[ci2][:, ot2 * 128:(ot2 + 1) * 128],
                                     rhs=bvpb_sb[:, ci2:ci2 + 1],
                                     start=(ci2 == 0), stop=(ci2 == NT - 1))
                nc.vector.tensor_add(out=bop_sb[:, oc2], in0=bps3[:, :],
                                     in1=bov_sb[:, oc2])
        if jt >= 16 and jt % 4 == 0:
            # residual' = x_resid + bo'' (spread so DVE never backlogs)
            co2 = (jt - 16) // 4
            nc.vector.tensor_scalar_add(out=xr_sb[co2], in0=xr_sb[co2],
                                        scalar1=bop_sb[:, co2:co2 + 1])

    # ---- attention chunks (fp8 DoubleRow logits + o) ----------------------
    pp = tc.alloc_tile_pool(name="pp", bufs=2)
    osb = tc.alloc_tile_pool(name="osb", bufs=4)
    outp = tc.alloc_tile_pool(name="outp", bufs=4)
    smsb = tc.alloc_tile_pool(name="smsb", bufs=1)

    out_r = d["out"].rearrange("(t p) n -> p t n", p=128)
    for ch in range(NCH):
        csl = slice(ch * CW, (ch + 1) * CW)
        o_ps = [ps_o.tile([128, CW], F32, tag="o", name=f"o{ch}_{i}") for i in range(4)]
        sacc = smsb.tile([128, CW], F32R, tag="sacc", name=f"sacc{ch}")
        Pt = None
        for jt in range(JT):
            jsl = slice(jt * 128, (jt + 1) * 128)
            lps = ps_mm.tile([128, CW], F32, tag="mm")
            for g in range(NG):
                nc.tensor.matmul(out=lps[:, :],
                                 lhsT=_pairs(x8_sb[g])[:, :, jsl],
                                 rhs=_pairs(qk_sb[ch][g])[:, :, :],
                                 start=(g == 0), stop=(g == NG - 1),
                                 perf_mode=DR)
            if jt % 2 == 0:
                Pt = pp.tile([128, 2 * CW], FP8, tag="P")
            Ps = Pt[:, (jt % 2) * CW:(jt % 2 + 1) * CW]
            nc.scalar.activation(out=Ps, in_=lps[:, :], func=AF.Exp,
                                 bias=eshift_sb[:, :], scale=SCALE)
            if jt % 2:
                jp = jt // 2
                for co in range(4):
                    nc.tensor.matmul(out=o_ps[co][:, :],
                                     lhsT=_pairs(vt_sb[jp])[:, :,
                                                            co * 128:(co + 1) * 128],
                                     rhs=_pairs(Pt)[:, :, :],
                                     start=(jp == 0), stop=(jp == JP - 1),
                                     perf_mode=DR, skip_group_check=True)
            if jt == 0:
                nc.vector.tensor_copy(out=sacc[:, :], in_=Ps)
            else:
                nc.vector.tensor_add(out=sacc[:, :], in0=_f32(sacc[:, :]), in1=Ps)

        # epilogue: 1/sums; last chunk normalizes during the o copy so the
        # final chain is one add per co (residual' already carries bo'').
        last = ch == NCH - 1
        rbp = ps_mm.tile([128, CW], F32, tag="mm")
        nc.tensor.matmul(out=rbp[:, :], lhsT=ones128_sb[:, :], rhs=sacc[:, :],
                         start=True, stop=True)
        rsb = smsb.tile([128, CW], F32, tag="rsb")
        nc.vector.reciprocal_approx_fast(out=rsb[:, :], in_=rbp[:, :])
        o_sb = []
        for co in range(4):
            ot_ = osb.tile([128, CW], BF16, tag="osb")
            if co % 2:
                nc.scalar.copy(out=ot_[:, :], in_=o_ps[co][:, :])
            else:
                nc.vector.tensor_copy(out=ot_[:, :], in_=o_ps[co][:, :])
            o_sb.append(ot_)
        for co in range(4):
            prp = ps_o.tile([128, CW], F32, tag="o", name=f"pr{ch}_{co}")
            for c in range(NT):
                nc.tensor.matmul(out=prp[:, :],
                                 lhsT=woT_sb[c][:, co * 128:(co + 1) * 128],
                                 rhs=o_sb[c][:, :],
                                 start=(c == 0), stop=(c == NT - 1))
            ou = outp.tile([128, CW], F32, tag="out")
            nc.vector.tensor_mul(out=ou[:, :], in0=prp[:, :], in1=rsb[:, :])
            nc.vector.tensor_add(out=ou[:, :], in0=ou[:, :],
                                 in1=xr_sb[co][:, csl])
            nc.sync.dma_start(out=out_r[:, co, csl], in_=ou[:, :])

    for p in (smsb, outp, osb, pp, ps_x, ps_o, ps_mm, vtp, qkp, vecs, mp,
              wv8p, wbp, rp, xp):
        p.release()


def _sel_consts():
    sel = np.zeros((128, GPT), np.float32)
    for p in range(128):
        sel[p, p // 16] = 1.0
    return sel, np.ascontiguousarray(sel.T)


def _pair_pack(a):
    """[C, M] -> [2*128, 2*M] fp8 pair layout: row g*128+p, col s*M+m."""
    Cr, M = a.shape
    f8 = ml_dtypes.float8_e4m3
    return np.ascontiguousarray(
        a.reshape(2, 2, 128, M).transpose(0, 2, 1, 3).reshape(2 * 128, 2 * M)
    ).astype(f8)


def kernel(x, gn_w, gn_b, wq, bq, wk, bk, wv, bv, wo, bo):
    del bk  # exactly cancelled by softmax shift invariance
    if "nc" not in _CACHE:
        _CACHE["nc"] = _build_bass()
    nc = _CACHE["nc"]

    bf = ml_dtypes.bfloat16
    x = np.ascontiguousarray(np.asarray(x, np.float32)).reshape(B, C, N)
    wb = np.ascontiguousarray(np.concatenate(
        [np.asarray(wq, np.float32), np.asarray(wk, np.float32),
         np.asarray(wo, np.float32).T], axis=1).astype(bf))
    wv8 = _pair_pack(np.ascontiguousarray(np.asarray(wv, np.float32).T))
    sel, selT = _sel_consts()
    vb = np.empty((128, 28), np.float32)
    for i, v in enumerate((gn_w, gn_b, bv, bo, bq)):
        vb[:, i * NT:(i + 1) * NT] = np.asarray(v, np.float32).reshape(NT, 128).T
    vb[:, 20:28] = sel

    in_maps = []
    for core in range(8):
        b, qb = core // 4, core % 4
        xb = np.ascontiguousarray(np.roll(x[b], -qb * NQ, axis=1))
        xr = np.ascontiguousarray(
            xb[:, :NQ].reshape(NT, 128, NQ).transpose(1, 0, 2)
            .reshape(128, NT * NQ).astype(ml_dtypes.bfloat16))
        in_maps.append({"xp8": _pair_pack(xb), "xr": xr,
                        "wb": wb, "wv8": wv8, "vb": vb, "selT": selT})

    _CACHE["last_in_maps"] = in_maps
    res = run_bass_kernel_spmd(nc, in_maps, list(range(8))).results
    out = np.empty((B, C, N), np.float32)
    for core in range(8):
        b, qb = core // 4, core % 4
        out[b][:, qb * NQ:(qb + 1) * NQ] = res[core]["out"]
    return out.reshape(B, C, HH, WW)


# revision 66
# speedup vs baseline: 1.0407x; 1.0407x over previous
"""AttnBlock (GroupNorm + single-head self-attention + proj + residual) on 8 trn2 cores.

Sharding: core = (batch b = core//4, query-block qb = core%4). Each core gets its
batch's x rolled so its 1024 queries are columns 0:1024; attention key/value
order is permutation-invariant so the roll is free. No cross-core communication.

Math (v6, fp8 DoubleRow attention core):
  GroupNorm folded into per-channel affine A, B: hn = A*x + B.
  Logits fold ("M-trick"): logits[j,i] = x[:,j]^T A (wk^T wq) A x[:,i] + gamma[j]
    MTraw = wq^T wk (bf16, weights only -- computed during the x DMA);
    MT1 = A_q*MTraw rounded to fp8 pairs; qk = A_k*(MT1^T-pairs @ x_fp8) + A_k*c0
    c0[ck] = sum_cq MTraw[cq,ck] B[cq] + (wk^T bq)[ck] (k-bias drops by softmax
    shift invariance; the q-bias survives as this per-key offset).
  P = exp(logits/sqrt(C) - 2) unnormalized (the -2 keeps P <= ~40, safely inside
  fp8e4m3 range; the shift cancels in the normalization). o = (wv*A@x) @ P with
  the division by column sums applied at the projection output.
  v/o biases collapse to bo'' = wo@(wv@B + bv) + bo, pre-added to the residual.

fp8 DoubleRow: the four big matmul streams (qk, vT, logits, o) run with
e4m3 operands and perf_mode=DoubleRow -- 256-deep contraction per call at 0.5
PE cycles/row, halving tensor-engine time vs bf16. Pair layout is slot-major
blocks ([p, (two n)]), so every producer (ACT exp, DVE scalar ops) writes a
contiguous slab; no extra interleave passes exist anywhere.
Accuracy: M/proj matmuls stay bf16; all PSUM accumulation f32; softmax sums
f32; residual f32. The fp8 rounding lands on tensors whose error averages out
over the 4096-key softmax (measured: see test.py).

Schedule: 11 DMA descriptors (issue cost ~620ns each on Sync), ordered
x_fp8 -> wq|wk|woT -> vecs -> wv_fp8 -> residual. GroupNorm stats split
DVE (bn_stats, 5/8) vs ACT (Square/Copy+accum, 3/8). Warm matmuls bridge the
two DMA waits so the HAM clock never drops (idle >2us halves the PE clock for
~20us). vT-phase PSUM->SBUF copies alternate DVE/ACT.
"""

import numpy as np
import ml_dtypes

import concourse.bass as bass
import concourse.bacc as bacc
import concourse.tile as tile
from concourse import mybir
from concourse.bass_utils import run_bass_kernel_spmd

F32 = mybir.dt.float32
F32R = mybir.dt.float32r
BF16 = mybir.dt.bfloat16
FP8 = mybir.dt.float8e4
AF = mybir.ActivationFunctionType
ALU = mybir.AluOpType
AX = mybir.AxisListType
DR = mybir.MatmulPerfMode.DoubleRow

B, C, HH, WW = 2, 512, 64, 64
N = HH * WW          # 4096 pixels
NQ = N // 4          # queries per core
G = 32               # groups
GPT = 8              # groups per 128-channel tile
NT = C // 128        # 4 channel blocks
NG = 2               # fp8 pair groups (256 channels each)
JT = N // 128        # 32 key tiles
JP = JT // 2         # 16 key pair-tiles
CW = 512             # query chunk width
NCH = NQ // CW       # 2 chunks per core
EPS = 1e-6
SCALE = float(C) ** -0.5
ESHIFT = -2.0        # exp(s*logit + ESHIFT): keeps P within fp8e4m3 range
BSC = 16.0           # B is scaled by 16 into fp8 so it sits in normal range

_CACHE: dict = {}


def _f32(ap):
    return ap.bitcast(F32)


def _pairs(tile_ap):
    return tile_ap.rearrange("p (two n) -> p two n", two=2)


def _build_bass():
    nc = bacc.Bacc("TRN2")

    xp8_d = nc.declare_dram_parameter("xp8", [2 * 128, 2 * N], FP8, isOutput=False)
    wb_d = nc.declare_dram_parameter("wb", [C, 3 * C], BF16, isOutput=False)
    vb_d = nc.declare_dram_parameter("vb", [128, 28], F32, isOutput=False)
    selT_d = nc.declare_dram_parameter("selT", [GPT, 128], F32, isOutput=False)
    wv8_d = nc.declare_dram_parameter("wv8", [2 * 128, 2 * C], FP8, isOutput=False)
    xr_d = nc.declare_dram_parameter("xr", [128, NT * NQ], BF16, isOutput=False)
    out_d = nc.declare_dram_parameter("out", [C, NQ], F32, isOutput=True)

    dram = dict(xp8=xp8_d, wb=wb_d, vb=vb_d, selT=selT_d, wv8=wv8_d, xr=xr_d,
                out=out_d)
    with tile.TileContext(nc) as tc, \
         nc.allow_low_precision(reason="fp8/bf16 PE inputs with f32 accumulation"):
        _emit(tc, {k: v.ap() for k, v in dram.items()})
    nc.compile()
    return nc


def _emit(tc, d):
    nc = tc.nc

    # ---- long-lived pools -------------------------------------------------
    xp = tc.alloc_tile_pool(name="xp", bufs=NG)
    rp = tc.alloc_tile_pool(name="rp", bufs=1)
    wbp = tc.alloc_tile_pool(name="wbp", bufs=NT)
    wv8p = tc.alloc_tile_pool(name="wv8p", bufs=NG)
    mp = tc.alloc_tile_pool(name="mp", bufs=NT + NG)  # MTraw bf16 + MT1 fp8
    vecs = tc.alloc_tile_pool(name="vecs", bufs=1)
    qkp = tc.alloc_tile_pool(name="qkp", bufs=NCH * NG)
    vtp = tc.alloc_tile_pool(name="vtp", bufs=JP)

    # ---- DMA in (11 issues; issue order ~ arrival order) ------------------
    # x in fp8 pair layout: tile g row p col (s*N + n) = x[256g+128s+p, n]
    # one DMA per slab (slot-major half): per-queue bandwidth is the limiter,
    # and the finer grain lets each slab's stats start on its own completion
    xg = d["xp8"].rearrange("(g p) m -> g p m", p=128)
    x8_sb = []
    for g in range(NG):
        xt = xp.tile([128, 2 * N], FP8, tag="x8", name=f"x8_{g}")
        for s in range(2):
            nc.sync.dma_start(out=xt[:, s * N:(s + 1) * N],
                              in_=xg[g][:, s * N:(s + 1) * N])
        x8_sb.append(xt)

    vb_sb = vecs.tile([128, 28], F32, tag="vb")
    nc.sync.dma_start(out=vb_sb[:, :], in_=d["vb"])
    gnw_sb = vb_sb[:, 0:4]
    gnb_sb = vb_sb[:, 4:8]
    bvv_sb = vb_sb[:, 8:12]
    bov_sb = vb_sb[:, 12:16]
    bqf_sb = vb_sb[:, 16:20]
    sel_sb = vb_sb[:, 20:28]
    selT_sb = vecs.tile([GPT, 128], F32, tag="selT")
    nc.sync.dma_start(out=selT_sb[:, :], in_=d["selT"])

    wb_sb = [wbp.tile([128, 3 * C], BF16, tag="wb", name=f"wb{t}")
             for t in range(NT)]
    wq_sb = [w[:, 0:C] for w in wb_sb]
    wk_sb = [w[:, C:2 * C] for w in wb_sb]
    woT_sb = [w[:, 2 * C:3 * C] for w in wb_sb]
    wv8_sb = [wv8p.tile([128, 2 * C], FP8, tag="wv8", name=f"wv8_{g}")
              for g in range(NG)]
    xrt = rp.tile([128, NT * NQ], BF16, tag="xr")
    xr_sb = [xrt[:, t * NQ:(t + 1) * NQ] for t in range(NT)]

    # ---- SBUF constants ---------------------------------------------------
    warm_sb = vecs.tile([128, CW], BF16, tag="warm")
    nc.vector.memset(warm_sb[:, :], 0.0)
    ones32_sb = vecs.tile([128, 128], F32, tag="ones32")
    nc.vector.memset(ones32_sb[:, :], 1.0)
    ones128_sb = vecs.tile([128, 128], F32R, tag="ones128")
    nc.vector.tensor_copy(out=ones128_sb[:, :], in_=ones32_sb[:, :])
    eshift_sb = vecs.tile([128, 1], F32, tag="eshift")
    nc.vector.memset(eshift_sb[:, :], ESHIFT)

    # (measured: x's slab arrival is a DMA-engine floor, ~0.2MB/us for the
    # fp8 stream regardless of contention — gating the transfers below on
    # x's first slab gained nothing and delayed the weights)
    wb_t = d["wb"].rearrange("(t p) m -> t p m", p=128)
    for t in range(NT):
        nc.sync.dma_start(out=wb_sb[t][:, :], in_=wb_t[t])
    wv8g = d["wv8"].rearrange("(g p) m -> g p m", p=128)
    for g in range(NG):
        nc.sync.dma_start(out=wv8_sb[g][:, :], in_=wv8g[g])
    nc.sync.dma_start(out=xrt[:, :], in_=d["xr"])

    A_sb = vecs.tile([128, NT], F32, tag="A")
    B_sb = vecs.tile([128, NT], F32, tag="B")
    Bb_sb = vecs.tile([128, NT], BF16, tag="Bb")      # B bf16 for c0 rhs
    B8_sb = vecs.tile([128, NT], FP8, tag="B8")       # 16*B fp8 for bv' rhs
    bqv_sb = vecs.tile([128, NT], BF16, tag="bqv")    # bq bf16
    c0A_sb = vecs.tile([128, NT], F32, tag="c0A")
    bvp_sb = vecs.tile([128, NT], F32, tag="bvp")
    bvpb_sb = vecs.tile([128, NT], BF16, tag="bvpb")
    bop_sb = vecs.tile([128, NT], F32, tag="bop")
    nc.vector.tensor_copy(out=bqv_sb[:, :], in_=bqf_sb)

    # ---- prologue: warm-up + M_raw + GroupNorm stats ----------------------
    with tc.tile_pool(name="stp", bufs=4) as stp, \
         tc.tile_pool(name="pspro", bufs=2, space="PSUM") as ps_pro:
        nwarm = [0]

        def emit_warm(n):
            for _ in range(n):
                wt = ps_pro.tile([128, CW], F32, tag="pro", name=f"wm{nwarm[0]}")
                nwarm[0] += 1
                nc.tensor.matmul(out=wt[:, :], lhsT=warm_sb[:, 0:128],
                                 rhs=warm_sb[:, :], start=True, stop=True)

        emit_warm(46)

        # MTraw[cq, ck] = sum_o wq[o, cq] wk[o, ck]  (weights only, no x/A dep)
        mt_sb = []
        for cq in range(NT):
            mps = ps_pro.tile([128, C], F32, tag="pro", name=f"mps{cq}")
            for ot in range(NT):
                nc.tensor.matmul(out=mps[:, :],
                                 lhsT=wq_sb[ot][:, cq * 128:(cq + 1) * 128],
                                 rhs=wk_sb[ot][:, :],
                                 start=(ot == 0), stop=(ot == NT - 1))
            mt = mp.tile([128, C], BF16, tag="mt", name=f"mt{cq}")
            nc.scalar.copy(out=mt[:, :], in_=mps[:, :])
            mt_sb.append(mt)

        emit_warm(10)

        # stats per channel-block slab (g, s) as the x DMA lands, split:
        #   segs 0-4: DVE bn_stats -> [mean_a, var_a]
        #   segs 5-7: ACT Square+accum (sumsq_b), ACT Copy+accum (sum_b)
        # st2 rows: [mean_a, E2_a, sum_b, sumsq_b]; weights applied after the
        # group-select matmul on the [8,4] tile.
        NSA, NSB = 5 * 512, 3 * 512
        sqs = vecs.tile([128, NSB], BF16, tag="sqs")   # ACT scratch
        gps_t = []
        for bi in range(NT):
            g, s = bi // 2, bi % 2
            slab = x8_sb[g][:, s * N:(s + 1) * N]
            st = stp.tile([128, 5, 6], F32, tag="bnst", name=f"bnst{bi}")
            xr_ = slab[:, 0:NSA].rearrange("p (s n) -> p s n", s=5)
            st2 = stp.tile([128, 4], F32, tag="st2", name=f"st2_{bi}")
            nc.scalar.activation(out=sqs[:, :], in_=slab[:, NSA:N],
                                 func=AF.Square, bias=0.0, scale=1.0,
                                 accum_out=st2[:, 3:4])
            nc.scalar.activation(out=sqs[:, :], in_=slab[:, NSA:N],
                                 func=AF.Copy, scale=1.0,
                                 accum_out=st2[:, 2:3])
            for q in range(5):
                nc.vector.bn_stats(out=st[:, q, :], in_=xr_[:, q, :])
            mv = stp.tile([128, 2], F32, tag="mv", name=f"mv{bi}")
            nc.vector.bn_aggr(out=mv[:, :], in_=st[:, :, :])
            nc.vector.tensor_copy(out=st2[:, 0:1], in_=mv[:, 0:1])
            nc.vector.tensor_mul(out=st2[:, 1:2], in0=mv[:, 0:1], in1=mv[:, 0:1])
            nc.vector.tensor_add(out=st2[:, 1:2], in0=st2[:, 1:2], in1=mv[:, 1:2])
            gps = ps_pro.tile([GPT, 4], F32, tag="gps", name=f"gps{bi}")
            nc.tensor.matmul(out=gps[:, :], lhsT=sel_sb, rhs=st2[:, :],
                             start=True, stop=True)
            gps_t.append(gps)
            emit_warm((3, 3, 3, 0)[bi])

        # group mean / rstd; DVE preps first, then batched ACT Sqrts
        GN = 16.0 * N
        grp_t = []
        for t in range(NT):
            grp = stp.tile([GPT, 2], F32, tag="grp", name=f"grp{t}")
            ga = stp.tile([GPT, 2], F32, tag="ga", name=f"ga{t}")
            nc.vector.tensor_scalar_mul(out=ga[:, :], in0=gps_t[t][:, 0:2],
                                        scalar1=NSA / GN)
            nc.vector.tensor_scalar_mul(out=grp[:, :], in0=gps_t[t][:, 2:4],
                                        scalar1=1.0 / GN)
            nc.vector.tensor_add(out=grp[:, :], in0=grp[:, :], in1=ga[:, :])
            gtmp = stp.tile([GPT, 1], F32, tag="gtmp", name=f"gtmp{t}")
            nc.vector.tensor_mul(out=gtmp[:, :], in0=grp[:, 0:1], in1=grp[:, 0:1])
            nc.vector.tensor_sub(out=grp[:, 1:2], in0=grp[:, 1:2], in1=gtmp[:, :])
            nc.vector.tensor_scalar_add(out=grp[:, 1:2], in0=grp[:, 1:2], scalar1=EPS)
            grp_t.append(grp)
        for t in range(NT):
            nc.scalar.activation(out=grp_t[t][:, 1:2], in_=grp_t[t][:, 1:2],
                                 func=AF.Sqrt, bias=0.0, scale=1.0)
        mt1_sb = []
        for t in range(NT):
            nc.vector.reciprocal(out=grp_t[t][:, 1:2], in_=grp_t[t][:, 1:2])
            mrp = ps_pro.tile([128, 2], F32, tag="sm", name=f"mrp{t}")
            nc.tensor.matmul(out=mrp[:, :], lhsT=selT_sb[:, :], rhs=grp_t[t][:, :],
                             start=True, stop=True)
            tcol = slice(t, t + 1)
            nc.vector.tensor_mul(out=A_sb[:, tcol], in0=gnw_sb[:, tcol], in1=mrp[:, 1:2])
            nc.vector.tensor_mul(out=B_sb[:, tcol], in0=mrp[:, 0:1], in1=A_sb[:, tcol])
            nc.vector.tensor_sub(out=B_sb[:, tcol], in0=gnb_sb[:, tcol], in1=B_sb[:, tcol])
            nc.vector.tensor_copy(out=Bb_sb[:, tcol], in_=B_sb[:, tcol])
            nc.vector.tensor_scalar_mul(out=B8_sb[:, tcol], in0=B_sb[:, tcol],
                                        scalar1=BSC)
            if t % 2:
                # MT1 fp8 pair g=t//2 as soon as its A columns exist, so only
                # the last pair's conversion sits on the A-critical chain
                g = t // 2
                m1 = mp.tile([128, 2 * C], FP8, tag="mt1", name=f"mt1_{g}")
                for s in range(2):
                    cq = 2 * g + s
                    nc.vector.tensor_scalar_mul(out=m1[:, s * C:(s + 1) * C],
                                                in0=mt_sb[cq][:, :],
                                                scalar1=A_sb[:, cq:cq + 1])
                mt1_sb.append(m1)

    ps_mm = tc.alloc_tile_pool(name="psmm", bufs=3, space="PSUM")
    nwarm2 = [0]

    def emit_warm2(n):
        for _ in range(n):
            wt = ps_mm.tile([128, CW], F32, tag="mm", name=f"w2_{nwarm2[0]}")
            nwarm2[0] += 1
            nc.tensor.matmul(out=wt[:, :], lhsT=warm_sb[:, 0:128],
                             rhs=warm_sb[:, :], start=True, stop=True)

    emit_warm2(8)

    # ---- c0A[ck] = A_k * (sum_cq MTraw[cq,ck] B[cq] + wk^T bq) ------------
    for ck in range(NT):
        cps = ps_mm.tile([128, 1], F32, tag="mm", name=f"c0{ck}")
        for cq in range(NT):
            nc.tensor.matmul(out=cps[:, :],
                             lhsT=mt_sb[cq][:, ck * 128:(ck + 1) * 128],
                             rhs=Bb_sb[:, cq:cq + 1],
                             start=(cq == 0), stop=False)
        for ot in range(NT):
            nc.tensor.matmul(out=cps[:, :],
                             lhsT=wk_sb[ot][:, ck * 128:(ck + 1) * 128],
                             rhs=bqv_sb[:, ot:ot + 1],
                             start=False, stop=(ot == NT - 1))
        nc.vector.tensor_mul(out=c0A_sb[:, ck:ck + 1], in0=cps[:, :],
                             in1=A_sb[:, ck:ck + 1])

    # ---- qk (both chunks, fp8 DoubleRow): A_k*(MT1^T @ x) + c0A -----------
    emit_warm2(8)
    qk_sb = [[None] * NG for _ in range(NCH)]
    for ch in range(NCH):
        csl = slice(ch * CW, (ch + 1) * CW)
        for ck in range(NT):
            kps = ps_mm.tile([128, CW], F32, tag="mm")
            for g in range(NG):
                nc.tensor.matmul(out=kps[:, :],
                                 lhsT=_pairs(mt1_sb[g])[:, :, ck * 128:(ck + 1) * 128],
                                 rhs=_pairs(x8_sb[g])[:, :, csl],
                                 start=(g == 0), stop=(g == NG - 1),
                                 perf_mode=DR)
            if qk_sb[ch][ck // 2] is None:
                qk_sb[ch][ck // 2] = qkp.tile([128, 2 * CW], FP8, tag="qk",
                                              name=f"qk{ch}_{ck // 2}")
            nc.vector.tensor_scalar(out=qk_sb[ch][ck // 2][:, (ck % 2) * CW:
                                                           (ck % 2 + 1) * CW],
                                    in0=kps[:, :],
                                    scalar1=A_sb[:, ck:ck + 1],
                                    scalar2=c0A_sb[:, ck:ck + 1],
                                    op0=ALU.mult, op1=ALU.add)

    # ---- folded v bias: bv' = wv@(16B)/16 + bv (unscaled fp8 wvT) ---------
    for ot in range(NT):
        ocol = slice(ot, ot + 1)
        bps2 = ps_mm.tile([128, 1], F32, tag="mm", name=f"bv{ot}")
        k = 0
        for g in range(NG):
            for s in range(2):
                nc.tensor.matmul(out=bps2[:, :],
                                 lhsT=wv8_sb[g][:, s * C + ot * 128:
                                                s * C + (ot + 1) * 128],
                                 rhs=B8_sb[:, 2 * g + s:2 * g + s + 1],
                                 start=(k == 0), stop=(k == 3))
                k += 1
        nc.vector.tensor_scalar(out=bvp_sb[:, ocol], in0=bps2[:, :],
                                scalar1=1.0 / BSC, scalar2=bvv_sb[:, ocol],
                                op0=ALU.mult, op1=ALU.add)
        nc.vector.tensor_copy(out=bvpb_sb[:, ocol], in_=bvp_sb[:, ocol])

    # ---- scale wv8 slabs by A (after the bias fold) -----------------------
    for g in range(NG):
        for s in range(2):
            nc.vector.tensor_scalar_mul(out=wv8_sb[g][:, s * C:(s + 1) * C],
                                        in0=wv8_sb[g][:, s * C:(s + 1) * C],
                                        scalar1=A_sb[:, 2 * g + s:2 * g + s + 1])

    ps_o = tc.alloc_tile_pool(name="pso", bufs=4, space="PSUM")
    # the 8th PSUM bank: a 4th rotating vT buffer so the PE never stalls on
    # the DVE/ACT vt-copy backlog (measured ~1.2us gap -> 6.8us half-clock)
    ps_x = tc.alloc_tile_pool(name="psx", bufs=1, space="PSUM")

    # ---- vT pairs: vt[jp] slot jt%2 = ((wv*A) @ x)^T[jt-block] ------------
    vt_sb = []
    for jt in range(JT):
        jsl = slice(jt * 128, (jt + 1) * 128)
        pool = ps_x if jt % 4 == 3 else ps_mm
        vps = pool.tile([128, C], F32, tag="mm" if pool is ps_mm else "vx",
                        name=f"vps{jt}")
        for g in range(NG):
            nc.tensor.matmul(out=vps[:, :],
                             lhsT=_pairs(x8_sb[g])[:, :, jsl],
                             rhs=_pairs(wv8_sb[g])[:, :, :],
                             start=(g == 0), stop=(g == NG - 1),
                             perf_mode=DR)
        if jt % 2 == 0:
            vt_sb.append(vtp.tile([128, 2 * C], FP8, tag="vt",
                                  name=f"vt{jt // 2}"))
        dst = vt_sb[jt // 2][:, (jt % 2) * C:(jt % 2 + 1) * C]
        if jt % 2:
            nc.scalar.copy(out=dst, in_=vps[:, :])
        else:
            nc.vector.tensor_copy(out=dst, in_=vps[:, :])
        if jt == 1:
            # bo'' = wo@bv' + bo, off the critical path once woT has landed
            for ot2 in range(NT):
                oc2 = slice(ot2, ot2 + 1)
                bps3 = ps_mm.tile([128, 1], F32, tag="mm", name=f"bo{ot2}")
                for ci2 in range(NT):
                    nc.tensor.matmul(out=bps3[:, :],
                                     lhsT=woT_sb[ci2][:, ot2 * 128:(ot2 + 1) * 128],
                                     rhs=bvpb_sb[:, ci2:ci2 + 1],
                                     start=(ci2 == 0), stop=(ci2 == NT - 1))
                nc.vector.tensor_add(out=bop_sb[:, oc2], in0=bps3[:, :],
                                     in1=bov_sb[:, oc2])
        if jt >= 16 and jt % 4 == 0:
            # residual' = x_resid + bo'' (spread so DVE never backlogs)
            co2 = (jt - 16) // 4
            nc.vector.tensor_scalar_add(out=xr_sb[co2], in0=xr_sb[co2],
                                        scalar1=bop_sb[:, co2:co2 + 1])

    # ---- attention chunks (fp8 DoubleRow logits + o) ----------------------
    pp = tc.alloc_tile_pool(name="pp", bufs=2)
    osb = tc.alloc_tile_pool(name="osb", bufs=4)
    outp = tc.alloc_tile_pool(name="outp", bufs=4)
    smsb = tc.alloc_tile_pool(name="smsb", bufs=1)

    out_r = d["out"].rearrange("(t p) n -> p t n", p=128)
    for ch in range(NCH):
        csl = slice(ch * CW, (ch + 1) * CW)
        o_ps = [ps_o.tile([128, CW], F32, tag="o", name=f"o{ch}_{i}") for i in range(4)]
        sacc = smsb.tile([128, CW], F32R, tag="sacc", name=f"sacc{ch}")
        Pt = None
        for jt in range(JT):
            jsl = slice(jt * 128, (jt + 1) * 128)
            lps = ps_mm.tile([128, CW], F32, tag="mm")
            for g in range(NG):
                nc.tensor.matmul(out=lps[:, :],
                                 lhsT=_pairs(x8_sb[g])[:, :, jsl],
                                 rhs=_pairs(qk_sb[ch][g])[:, :, :],
                                 start=(g == 0), stop=(g == NG - 1),
                                 perf_mode=DR)
            if jt % 2 == 0:
                Pt = pp.tile([128, 2 * CW], FP8, tag="P")
            Ps = Pt[:, (jt % 2) * CW:(jt % 2 + 1) * CW]
            nc.scalar.activation(out=Ps, in_=lps[:, :], func=AF.Exp,
                                 bias=eshift_sb[:, :], scale=SCALE)
            if jt % 2:
                jp = jt // 2
                for co in range(4):
                    nc.tensor.matmul(out=o_ps[co][:, :],
                                     lhsT=_pairs(vt_sb[jp])[:, :,
                                                            co * 128:(co + 1) * 128],
                                     rhs=_pairs(Pt)[:, :, :],
                                     start=(jp == 0), stop=(jp == JP - 1),
                                     perf_mode=DR, skip_group_check=True)
            if jt == 0:
                nc.vector.tensor_copy(out=sacc[:, :], in_=Ps)
            else:
                nc.vector.tensor_add(out=sacc[:, :], in0=_f32(sacc[:, :]), in1=Ps)

        # epilogue: 1/sums; last chunk normalizes during the o copy so the
        # final chain is one add per co (residual' already carries bo'').
        last = ch == NCH - 1
        rbp = ps_mm.tile([128, CW], F32, tag="mm")
        nc.tensor.matmul(out=rbp[:, :], lhsT=ones128_sb[:, :], rhs=sacc[:, :],
                         start=True, stop=True)
        rsb = smsb.tile([128, CW], F32, tag="rsb")
        nc.vector.reciprocal_approx_fast(out=rsb[:, :], in_=rbp[:, :])
        o_sb = []
        for co in range(4):
            ot_ = osb.tile([128, CW], BF16, tag="osb")
            if co % 2:
                nc.scalar.copy(out=ot_[:, :], in_=o_ps[co][:, :])
            else:
                nc.vector.tensor_copy(out=ot_[:, :], in_=o_ps[co][:, :])
            o_sb.append(ot_)
        for co in range(4):
            prp = ps_o.tile([128, CW], F32, tag="o", name=f"pr{ch}_{co}")
            for c in range(NT):
                nc.tensor.matmul(out=prp[:, :],
                                 lhsT=woT_sb[c][:, co * 128:(co + 1) * 128],
                                 rhs=o_sb[c][:, :],
                                 start=(c == 0), stop=(c == NT - 1))
            ou = outp.tile([128, CW], F32, tag="out")
            nc.vector.tensor_mul(out=ou[:, :], in0=prp[:, :], in1=rsb[:, :])
            nc.vector.tensor_add(out=ou[:, :], in0=ou[:, :],
                                 in1=xr_sb[co][:, csl])
            nc.sync.dma_start(out=out_r[:, co, csl], in_=ou[:, :])

    for p in (smsb, outp, osb, pp, ps_x, ps_o, ps_mm, vtp, qkp, vecs, mp,
              wv8p, wbp, rp, xp):
        p.release()


def _sel_consts():
    sel = np.zeros((128, GPT), np.float32)
    for p in range(128):
        sel[p, p // 16] = 1.0
    return sel, np.ascontiguousarray(sel.T)


def _pair_pack(a):
    """[C, M] -> [2*128, 2*M] fp8 pair layout: row g*128+p, col s*M+m."""
    Cr, M = a.shape
    f8 = ml_dtypes.float8_e4m3
    return np.ascontiguousarray(
        a.reshape(2, 2, 128, M).transpose(0, 2, 1, 3).reshape(2 * 128, 2 * M)
    ).astype(f8)


def kernel(x, gn_w, gn_b, wq, bq, wk, bk, wv, bv, wo, bo):
    del bk  # exactly cancelled by softmax shift invariance
    if "nc" not in _CACHE:
        _CACHE["nc"] = _build_bass()
    nc = _CACHE["nc"]

    bf = ml_dtypes.bfloat16
    x = np.ascontiguousarray(np.asarray(x, np.float32)).reshape(B, C, N)
    wb = np.ascontiguousarray(np.concatenate(
        [np.asarray(wq, np.float32), np.asarray(wk, np.float32),
         np.asarray(wo, np.float32).T], axis=1).astype(bf))
    wv8 = _pair_pack(np.ascontiguousarray(np.asarray(wv, np.float32).T))
    sel, selT = _sel_consts()
    vb = np.empty((128, 28), np.float32)
    for i, v in enumerate((gn_w, gn_b, bv, bo, bq)):
        vb[:, i * NT:(i + 1) * NT] = np.asarray(v, np.float32).reshape(NT, 128).T
    vb[:, 20:28] = sel

    in_maps = []
    for core in range(8):
        b, qb = core // 4, core % 4
        xb = np.ascontiguousarray(np.roll(x[b], -qb * NQ, axis=1))
        xr = np.ascontiguousarray(
            xb[:, :NQ].reshape(NT, 128, NQ).transpose(1, 0, 2)
            .reshape(128, NT * NQ).astype(ml_dtypes.bfloat16))
        in_maps.append({"xp8": _pair_pack(xb), "xr": xr,
                        "wb": wb, "wv8": wv8, "vb": vb, "selT": selT})

    _CACHE["last_in_maps"] = in_maps
    res = run_bass_kernel_spmd(nc, in_maps, list(range(8))).results
    out = np.empty((B, C, N), np.float32)
    for core in range(8):
        b, qb = core // 4, core % 4
        out[b][:, qb * NQ:(qb + 1) * NQ] = res[core]["out"]
    return out.reshape(B, C, HH, WW)
